# revision 8
# baseline (speedup 1.0000x reference)
"""MCRGANloss Trainium2 kernel — fully on-device (Grams + logdets).

Sharding: core c owns class c (padded to 32 tiles of 128 rows) plus a
quarter of a shared class (cores 0-3: class 8; cores 4-7: class 9),
padded to 8 tiles. Per-core 40 tiles for Z and Z_bar.

Device program (SPMD, static):
  1. Gram phase: two PSUM accumulation groups (own 32 tiles / shared 8
     tiles) x 2 tensors x 2 column halves, fp32r matmuls.
  2. Collectives: AllReduce shared-class Grams within [[0-3],[4-7]];
     AllReduce own-class and shared Grams over all 8 for the full Gram.
  3. Assemble 4 SPD matrices B_m = Gram-combo + (1/s) I per core.
  4. logdet each B_m: block-LDL at 128 with Newton-Schulz inverses;
     per-stage logdet of the 128x128 Schur block via inverse-cascade
     (two 32x32 pivot LDL loops on known blocks + two on Schur
     complements formed with warm-started NS-32 inverses).
  5. Output 4 logdets per core; host combines (adds d*log(s) terms).
"""

import numpy as np

EPS = 0.5
J = 10
N_CORES = 8
D = 1024
OWN_TILES = 32
SH_TILES = 8
CORE_TILES = OWN_TILES + SH_TILES
NS128_ITERS = 9
NS32_ITERS = 2

_cache = {}


def build_v2():
    import concourse.bass as bass
    import concourse.bacc as bacc
    import concourse.mybir as mybir
    from concourse import tile

    f32 = mybir.dt.float32
    f32r = mybir.dt.float32r
    AL = mybir.AluOpType
    AF = mybir.ActivationFunctionType

    nc = bacc.Bacc("TRN2", target_bir_lowering=False, debug=False,
                   num_devices=N_CORES)

    zt = nc.dram_tensor("zt", [CORE_TILES * 128, D], f32, kind="ExternalInput")
    zbt = nc.dram_tensor("zbt", [CORE_TILES * 128, D], f32, kind="ExternalInput")
    # consts / per-core params (all [128, x], replicated where scalar)
    ident = nc.dram_tensor("ident", [128, 128], f32, kind="ExternalInput")
    diags = nc.dram_tensor("diags", [128, 4 * 128], f32, kind="ExternalInput")
    wts = nc.dram_tensor("wts", [128, 4], f32, kind="ExternalInput")
    alphas = nc.dram_tensor("alphas", [128, 4], f32, kind="ExternalInput")
    lds_out = nc.dram_tensor("lds", [4, 1], f32, kind="ExternalOutput")

    with tile.TileContext(nc) as tc:
        with (
            tc.tile_pool(name="mats", bufs=1) as mpool,
            tc.tile_pool(name="dram", bufs=1, space="DRAM") as dpool,
            tc.tile_pool(name="cpool", bufs=1) as cpool,
        ):
            # 4 matrices, each [128, 8*1024] (row-block rb at cols rb*1024..)
            mats = [mpool.tile([128, 8 * 1024], f32, tag=f"mat{m}",
                               name=f"mat{m}") for m in range(4)]
            # DRAM bounces for collectives
            bA = dpool.tile([2 * D, D], f32, name="bA")
            bB = dpool.tile([2 * D, D], f32, name="bB")
            rB = dpool.tile([2 * D, D], f32, name="rB")
            rA = dpool.tile([2 * D, D], f32, name="rA")
            rBall = dpool.tile([2 * D, D], f32, name="rBall")

            idt = cpool.tile([128, 128], f32, name="idt")
            nc.sync.dma_start(idt[:], ident[:, :])
            i2 = cpool.tile([128, 128], f32, name="i2")
            nc.vector.tensor_scalar_mul(i2[:], idt[:], 2.0)
            dg = cpool.tile([128, 4 * 128], f32, name="dg")
            nc.sync.dma_start(dg[:], diags[:, :])
            wt = cpool.tile([128, 4], f32, name="wt")
            nc.sync.dma_start(wt[:], wts[:, :])
            alp = cpool.tile([128, 4], f32, name="alp")
            nc.sync.dma_start(alp[:], alphas[:, :])
            # weighted identities for B3 assembly
            wI = []
            for k in range(4):
                wik = cpool.tile([128, 128], f32, name=f"wI{k}")
                nc.vector.tensor_scalar_mul(wik[:], idt[:], wt[:, k:k + 1])
                wI.append(wik)

            # ---------------- Gram phase ----------------
            with (
                tc.tile_pool(name="gtiles", bufs=1) as tpool,
                tc.tile_pool(name="gstage", bufs=2) as spool,
                tc.tile_pool(name="gpsum", bufs=1, space="PSUM") as ppool,
            ):
                for ti, src in enumerate((zt, zbt)):
                    for half in range(2):
                        for grp, (t0, t1) in ((1, (OWN_TILES, CORE_TILES)),
                                              (0, (0, OWN_TILES))):
                            banks = [ppool.tile([128, 512], f32, tag=f"bank{m}",
                                                name=f"bank_{ti}_{half}_{grp}_{m}")
                                     for m in range(8)]
                            for t in range(t0, t1):
                                tl = tpool.tile([128, D], f32r,
                                                tag=f"in{t % 10}",
                                                name=f"in_{ti}_{half}_{t}")
                                nc.sync.dma_start(
                                    tl[:], src[t * 128:(t + 1) * 128, :].bitcast(f32r))
                                rhs = tl[:, half * 512:half * 512 + 512]
                                for m in range(8):
                                    nc.tensor.matmul(
                                        banks[m][:],
                                        tl[:, m * 128:(m + 1) * 128],
                                        rhs,
                                        start=(t == t0), stop=(t == t1 - 1),
                                        skip_group_check=True)
                            for m in range(8):
                                dst_col = m * 1024 + half * 512
                                if grp == 0:
                                    # own-class Gram -> mats[ti] directly
                                    if m % 2 == 0:
                                        nc.vector.tensor_copy(
                                            mats[ti][:, dst_col:dst_col + 512],
                                            banks[m][:])
                                    else:
                                        nc.scalar.copy(
                                            mats[ti][:, dst_col:dst_col + 512],
                                            banks[m][:])
                                else:
                                    st = spool.tile([128, 512], f32,
                                                    tag=f"st{m % 4}",
                                                    name=f"st_{ti}_{half}_{m}")
                                    if m % 2 == 0:
                                        nc.vector.tensor_copy(st[:], banks[m][:])
                                    else:
                                        nc.scalar.copy(st[:], banks[m][:])
                                    nc.sync.dma_start(
                                        bB[ti * D + m * 128:ti * D + m * 128 + 128,
                                           half * 512:half * 512 + 512], st[:])
                # own-class Grams -> bA for the F collective (pure Grams)
                for ti in range(2):
                    for rb in range(8):
                        nc.sync.dma_start(
                            bA[ti * D + rb * 128:ti * D + rb * 128 + 128, :],
                            mats[ti][:, rb * 1024:rb * 1024 + 1024])

            # ---------------- Collectives ----------------
            nc.gpsimd.collective_compute(
                "AllReduce", mybir.AluOpType.add,
                replica_groups=[[0, 1, 2, 3], [4, 5, 6, 7]],
                ins=[bB.opt()], outs=[rB.opt()])
            nc.gpsimd.collective_compute(
                "AllReduce", mybir.AluOpType.add,
                replica_groups=[list(range(8))],
                ins=[bA.opt()], outs=[rA.opt()])
            nc.gpsimd.collective_compute(
                "AllReduce", mybir.AluOpType.add,
                replica_groups=[list(range(8))],
                ins=[bB.opt()], outs=[rBall.opt()])

            # ---------------- Assembly of B2, B3 ----------------
            with (
                tc.tile_pool(name="atmp", bufs=4) as apool,
                tc.tile_pool(name="apsum", bufs=2, space="PSUM") as appool,
            ):
                # B2 = mat0 + mat1 (+ corrected diag later), via PE identity
                for rb in range(8):
                    for h in range(2):
                        col = rb * 1024 + h * 512
                        ps = appool.tile([128, 512], f32, tag="aps",
                                         name=f"b2ps_{rb}_{h}")
                        nc.tensor.matmul(ps[:], idt[:],
                                         mats[0][:, col:col + 512],
                                         start=True, stop=False,
                                         skip_group_check=True)
                        nc.tensor.matmul(ps[:], idt[:],
                                         mats[1][:, col:col + 512],
                                         start=False, stop=True,
                                         skip_group_check=True)
                        if h == 0:
                            nc.vector.tensor_copy(mats[2][:, col:col + 512], ps[:])
                        else:
                            nc.scalar.copy(mats[2][:, col:col + 512], ps[:])
                # B3 = w0*rB[Z] + w1*rB[Zb] + w2*(rA[Z]+rBall[Z]) + w3*(rA[Zb]+rBall[Zb])
                for rb in range(8):
                    for h in range(2):
                        col = rb * 1024 + h * 512
                        ps = appool.tile([128, 512], f32, tag="aps",
                                         name=f"b3ps_{rb}_{h}")
                        pieces = [(rB, 0, 0), (rB, 1, 1),
                                  (rA, 0, 2), (rBall, 0, 2),
                                  (rA, 1, 3), (rBall, 1, 3)]
                        for pi, (srcb, ti, k) in enumerate(pieces):
                            tmp = apool.tile([128, 512], f32, tag=f"at{pi % 4}",
                                             name=f"b3t_{rb}_{h}_{pi}")
                            nc.sync.dma_start(
                                tmp[:],
                                srcb[ti * D + rb * 128:ti * D + rb * 128 + 128,
                                     h * 512:h * 512 + 512])
                            nc.tensor.matmul(ps[:], wI[k][:],
                                             tmp[:],
                                             start=(pi == 0), stop=(pi == 5),
                                             skip_group_check=True)
                        if h == 0:
                            nc.vector.tensor_copy(mats[3][:, col:col + 512], ps[:])
                        else:
                            nc.scalar.copy(mats[3][:, col:col + 512], ps[:])
                # diag adds: B_m[rb-block diagonal 128-chunk] += diags[m]
                for m in range(4):
                    for rb in range(8):
                        col = rb * 1024 + rb * 128
                        nc.vector.tensor_add(
                            mats[m][:, col:col + 128],
                            mats[m][:, col:col + 128],
                            dg[:, m * 128:(m + 1) * 128])

            # ---------------- logdet phase ----------------
            with (
                tc.tile_pool(name="lwork", bufs=2) as lpool,
                tc.tile_pool(name="lpsum", bufs=2, space="PSUM") as lppool,
                tc.tile_pool(name="piv", bufs=1) as pvpool,
            ):
                pivs = pvpool.tile([128, 8 * 32 * 4], f32, name="pivs")
                for k in range(8):
                    cascb = pvpool.tile([128, 128], f32, tag="casc",
                                        bufs=2, name=f"casc_{k}")
                    for m in range(4):
                        mat = mats[m]

                        def blk(rb, c0, w):
                            return mat[:, rb * 1024 + c0:rb * 1024 + c0 + w]

                        S = blk(k, k * 128, 128)  # [128,128] diag block
                        # --- NS-128: X = inv(S) ---
                        X = lpool.tile([128, 128], f32, tag=f"X{m}",
                                       name=f"X_{k}_{m}")
                        nc.vector.tensor_scalar_mul(X[:], idt[:],
                                                    alp[:, m:m + 1])
                        for it in range(NS128_ITERS):
                            Yp = lppool.tile([128, 128], f32, tag="Yp",
                                             name=f"Yp_{k}_{m}_{it}")
                            nc.tensor.matmul(Yp[:], S, X[:], start=True,
                                             stop=True, skip_group_check=True)
                            T = lpool.tile([128, 128], f32, tag="T",
                                           name=f"T_{k}_{m}_{it}")
                            nc.vector.scalar_tensor_tensor(
                                T[:], Yp[:], -1.0, i2[:], AL.mult, AL.add)
                            X2 = lppool.tile([128, 128], f32, tag="Yp",
                                             name=f"X2_{k}_{m}_{it}")
                            nc.tensor.matmul(X2[:], X[:], T[:], start=True,
                                             stop=True, skip_group_check=True)
                            nc.scalar.copy(X[:], X2[:])

                        # --- panel + trailing update (stages < 7) ---
                        if k < 7:
                            wspan = (7 - k) * 128
                            rowp = blk(k, (k + 1) * 128, wspan)
                            Wt = lpool.tile([128, 896], f32, tag="Wt",
                                            name=f"Wt_{k}_{m}")
                            for c0 in range(0, wspan, 512):
                                w = min(512, wspan - c0)
                                Wp = lppool.tile([128, 512], f32, tag="Wp",
                                                 name=f"Wp_{k}_{m}_{c0}")
                                nc.tensor.matmul(Wp[:, :w], X[:],
                                                 rowp[:, c0:c0 + w],
                                                 start=True, stop=True,
                                                 skip_group_check=True)
                                nc.vector.tensor_scalar_mul(
                                    Wt[:, c0:c0 + w], Wp[:, :w], -1.0)
                            for ib in range(k + 1, 8):
                                wi = 1024 - 128 * ib
                                off = (ib - k - 1) * 128
                                tp = lppool.tile([128, 896], f32, tag="tp",
                                                 name=f"tp_{k}_{m}_{ib}")
                                for c0 in range(0, wi, 512):
                                    w = min(512, wi - c0)
                                    nc.tensor.matmul(
                                        tp[:, c0:c0 + w],
                                        Wt[:, off:off + 128],
                                        rowp[:, off + c0:off + c0 + w],
                                        start=True, stop=True,
                                        skip_group_check=True)
                                tgt = blk(ib, 128 * ib, wi)
                                nc.vector.tensor_tensor(
                                    tgt, tgt, tp[:, :wi], AL.add)

                        # --- cascade pieces into cascb[:, m*32:(m+1)*32] ---
                        cc = cascb[:, m * 32:(m + 1) * 32]
                        # (a) A11 = S[0:32,0:32]
                        nc.vector.tensor_copy(cc[0:32, :], S[0:32, 0:32])
                        # (c) XB11 = X[64:96,64:96]
                        nc.vector.tensor_copy(cc[64:96, :], X[64:96, 64:96])
                        # NS32 a: inv(A11), warm from X[0:32,0:32]
                        Xa = lpool.tile([32, 32], f32, tag="Xa",
                                        name=f"Xa_{k}_{m}")
                        nc.vector.tensor_copy(Xa[:], X[0:32, 0:32])
                        for it in range(NS32_ITERS):
                            yp = lppool.tile([32, 32], f32, tag="Yp",
                                             name=f"ya_{k}_{m}_{it}")
                            nc.tensor.matmul(yp[:], S[0:32, 0:32], Xa[:],
                                             start=True, stop=True,
                                             skip_group_check=True)
                            t3 = lpool.tile([32, 32], f32, tag="t3",
                                            name=f"ta_{k}_{m}_{it}")
                            nc.vector.scalar_tensor_tensor(
                                t3[:], yp[:], -1.0, i2[0:32, 0:32],
                                AL.mult, AL.add)
                            x2 = lppool.tile([32, 32], f32, tag="Yp",
                                             name=f"xa2_{k}_{m}_{it}")
                            nc.tensor.matmul(x2[:], Xa[:], t3[:], start=True,
                                             stop=True, skip_group_check=True)
                            nc.scalar.copy(Xa[:], x2[:])
                        # SchurA = S[32:64,32:64] - A21 Xa A12 -> cc[32:64]
                        t1p = lppool.tile([32, 32], f32, tag="Yp",
                                          name=f"t1a_{k}_{m}")
                        nc.tensor.matmul(t1p[:], Xa[:], S[0:32, 32:64],
                                         start=True, stop=True,
                                         skip_group_check=True)
                        t1s = lpool.tile([32, 32], f32, tag="t3",
                                         name=f"t1as_{k}_{m}")
                        nc.scalar.copy(t1s[:], t1p[:])
                        t2p = lppool.tile([128, 32], f32, tag="Yp",
                                          name=f"t2a_{k}_{m}")
                        nc.tensor.matmul(t2p[32:64, :], S[0:32, 32:64], t1s[:],
                                         start=True, stop=True,
                                         tile_position=(0, 32),
                                         skip_group_check=True)
                        nc.vector.scalar_tensor_tensor(
                            cc[32:64, :], t2p[32:64, :], -1.0, S[32:64, 32:64],
                            AL.mult, AL.add)
                        # NS32 b: inv(XB11), warm from S[64:96,64:96]
                        Xb = lpool.tile([128, 32], f32, tag="Xb",
                                        name=f"Xb_{k}_{m}")
                        nc.vector.tensor_copy(Xb[64:96, :], S[64:96, 64:96])
                        for it in range(NS32_ITERS):
                            yp = lppool.tile([128, 32], f32, tag="Yp",
                                             name=f"yb_{k}_{m}_{it}")
                            nc.tensor.matmul(yp[64:96, :], X[64:96, 64:96],
                                             Xb[64:96, :], start=True,
                                             stop=True, tile_position=(64, 64),
                                             skip_group_check=True)
                            t3 = lpool.tile([128, 32], f32, tag="t3b",
                                            name=f"tb_{k}_{m}_{it}")
                            nc.vector.scalar_tensor_tensor(
                                t3[64:96, :], yp[64:96, :], -1.0,
                                i2[64:96, 64:96], AL.mult, AL.add)
                            x2 = lppool.tile([128, 32], f32, tag="Yp",
                                             name=f"xb2_{k}_{m}_{it}")
                            nc.tensor.matmul(x2[64:96, :], Xb[64:96, :],
                                             t3[64:96, :], start=True,
                                             stop=True, tile_position=(64, 64),
                                             skip_group_check=True)
                            nc.scalar.copy(Xb[64:96, :], x2[64:96, :])
                        # SchurXB = X[96:128,96:128] - XB21 Xb XB12 -> cc[96:128]
                        u1p = lppool.tile([128, 32], f32, tag="Yp",
                                          name=f"u1_{k}_{m}")
                        nc.tensor.matmul(u1p[64:96, :], Xb[64:96, :],
                                         X[64:96, 96:128], start=True,
                                         stop=True, tile_position=(64, 64),
                                         skip_group_check=True)
                        u1s = lpool.tile([128, 32], f32, tag="t3b",
                                         name=f"u1s_{k}_{m}")
                        nc.scalar.copy(u1s[64:96, :], u1p[64:96, :])
                        u2p = lppool.tile([128, 32], f32, tag="Yp",
                                          name=f"u2_{k}_{m}")
                        nc.tensor.matmul(u2p[96:128, :], X[64:96, 96:128],
                                         u1s[64:96, :], start=True, stop=True,
                                         tile_position=(64, 96),
                                         skip_group_check=True)
                        nc.vector.scalar_tensor_tensor(
                            cc[96:128, :], u2p[96:128, :], -1.0,
                            X[96:128, 96:128], AL.mult, AL.add)

                    # --- batched pivot loop over cascb [128, 128] ---
                    b1 = pvpool.tile([128, 128], f32, tag="b1", name=f"b1_{k}")
                    b1t = pvpool.tile([128, 128], f32, tag="b1t",
                                      name=f"b1t_{k}")
                    wv = pvpool.tile([128, 4], f32, tag="wv", name=f"wv_{k}")
                    for j in range(32):
                        # v broadcast: b1[:, g*32+f] = cascb[:, g*32+j]
                        nc.vector.tensor_copy(
                            b1[:].rearrange("p (a b) -> p a b", a=4),
                            cascb[:, j::32].broadcast_to([128, 4, 32]))
                        nc.vector.transpose(b1t[:], b1[:])
                        # w = v / p  ([128,4] strided col slices)
                        vs = cascb[:, j::32]
                        ps_ = b1t[:, j::32]
                        nc.vector.reciprocal(wv[:], ps_)
                        nc.vector.tensor_tensor(wv[:], vs, wv[:], AL.mult)
                        # record pivots
                        nc.vector.tensor_copy(
                            pivs[:, (k * 32 + j) * 4:(k * 32 + j) * 4 + 4], ps_)
                        if j < 31:
                            # M = b1t * broadcast(w); cascb -= M
                            M = pvpool.tile([128, 128], f32, tag="Mt",
                                            name=f"M_{k}_{j}")
                            nc.vector.tensor_tensor(
                                M[:].rearrange("p (a b) -> p a b", a=4),
                                b1t[:].rearrange("p (a b) -> p a b", a=4),
                                wv[:].broadcast_to([128, 4, 32]), AL.mult)
                            nc.vector.tensor_tensor(cascb[:], cascb[:], M[:],
                                                    AL.subtract)

                # --- final: logs, sums, sign-combine, output ---
                lnp = pvpool.tile([128, 8 * 32 * 4], f32, name="lnp")
                nc.scalar.activation(lnp[:], pivs[:], AF.Ln)
                lnsum = pvpool.tile([128, 4], f32, name="lnsum")
                for m in range(4):
                    nc.vector.tensor_reduce(lnsum[:, m:m + 1],
                                            lnp[:, m::4],
                                            mybir.AxisListType.X, AL.add)
                tps = lppool.tile([4, 128], f32, tag="Wp", name="tps")
                nc.tensor.transpose(tps[:], lnsum[:], idt[:])
                tss = pvpool.tile([4, 128], f32, name="tss")
                nc.vector.tensor_copy(tss[:], tps[:])
                r1 = pvpool.tile([4, 1], f32, name="r1")
                r2 = pvpool.tile([4, 1], f32, name="r2")
                nc.vector.tensor_reduce(r1[:], tss[:, 0:64], mybir.AxisListType.X, AL.add)
                nc.vector.tensor_reduce(r2[:], tss[:, 64:128], mybir.AxisListType.X, AL.add)
                out4 = pvpool.tile([4, 1], f32, name="out4")
                nc.vector.tensor_tensor(out4[:], r1[:], r2[:], AL.subtract)
                nc.vector.tensor_scalar_mul(out4[:], out4[:], 1.0 / 32.0)
                nc.sync.dma_start(lds_out[:, :], out4[:])
    nc.compile()
    return nc


# revision 9
# speedup vs baseline: 1.0585x; 1.0585x over previous
"""MCRGANloss Trainium2 kernel — fully on-device (Grams + logdets).

Sharding: core c owns class c (padded to 32 tiles of 128 rows) plus a
quarter of a shared class (cores 0-3: class 8; cores 4-7: class 9),
padded to 8 tiles. Per-core 40 tiles for Z and Z_bar.

Device program (SPMD, static):
  1. Gram phase: two PSUM accumulation groups (own 32 tiles / shared 8
     tiles) x 2 tensors x 2 column halves, fp32r matmuls.
  2. Collectives: AllReduce shared-class Grams within [[0-3],[4-7]];
     AllReduce own-class and shared Grams over all 8 for the full Gram.
  3. Assemble 4 SPD matrices B_m = Gram-combo + (1/s) I per core.
  4. logdet each B_m: block-LDL at 128 with Newton-Schulz inverses;
     per-stage logdet of the 128x128 Schur block via inverse-cascade
     (two 32x32 pivot LDL loops on known blocks + two on Schur
     complements formed with warm-started NS-32 inverses).
  5. Output 4 logdets per core; host combines (adds d*log(s) terms).
"""

import numpy as np

EPS = 0.5
J = 10
N_CORES = 8
D = 1024
OWN_TILES = 32
SH_TILES = 8
CORE_TILES = OWN_TILES + SH_TILES
NS128_ITERS = 9
NS32_ITERS = 2

_cache = {}


def build_v2():
    import concourse.bass as bass
    import concourse.bacc as bacc
    import concourse.mybir as mybir
    from concourse import tile

    f32 = mybir.dt.float32
    f32r = mybir.dt.float32r
    AL = mybir.AluOpType
    AF = mybir.ActivationFunctionType

    nc = bacc.Bacc("TRN2", target_bir_lowering=False, debug=False,
                   num_devices=N_CORES)

    zt = nc.dram_tensor("zt", [CORE_TILES * 128, D], f32, kind="ExternalInput")
    zbt = nc.dram_tensor("zbt", [CORE_TILES * 128, D], f32, kind="ExternalInput")
    # consts / per-core params (all [128, x], replicated where scalar)
    ident = nc.dram_tensor("ident", [128, 128], f32, kind="ExternalInput")
    diags = nc.dram_tensor("diags", [128, 4 * 128], f32, kind="ExternalInput")
    wts = nc.dram_tensor("wts", [128, 4], f32, kind="ExternalInput")
    alphas = nc.dram_tensor("alphas", [128, 4], f32, kind="ExternalInput")
    lds_out = nc.dram_tensor("lds", [4, 1], f32, kind="ExternalOutput")

    with tile.TileContext(nc) as tc:
        with (
            tc.tile_pool(name="mats", bufs=1) as mpool,
            tc.tile_pool(name="dram", bufs=1, space="DRAM") as dpool,
            tc.tile_pool(name="cpool", bufs=1) as cpool,
        ):
            # 4 matrices, each [128, 8*1024] (row-block rb at cols rb*1024..)
            mats = [mpool.tile([128, 8 * 1024], f32, tag=f"mat{m}",
                               name=f"mat{m}") for m in range(4)]
            # DRAM bounces for collectives
            bA = dpool.tile([2 * D, D], f32, name="bA")
            bB = dpool.tile([2 * D, D], f32, name="bB")
            rB = dpool.tile([2 * D, D], f32, name="rB")
            rA = dpool.tile([2 * D, D], f32, name="rA")
            rBall = dpool.tile([2 * D, D], f32, name="rBall")

            idt = cpool.tile([128, 128], f32, name="idt")
            nc.sync.dma_start(idt[:], ident[:, :])
            i2 = cpool.tile([128, 128], f32, name="i2")
            nc.vector.tensor_scalar_mul(i2[:], idt[:], 2.0)
            dg = cpool.tile([128, 4 * 128], f32, name="dg")
            nc.sync.dma_start(dg[:], diags[:, :])
            wt = cpool.tile([128, 4], f32, name="wt")
            nc.sync.dma_start(wt[:], wts[:, :])
            alp = cpool.tile([128, 4], f32, name="alp")
            nc.sync.dma_start(alp[:], alphas[:, :])
            # weighted identities for B3 assembly
            wI = []
            for k in range(4):
                wik = cpool.tile([128, 128], f32, name=f"wI{k}")
                nc.vector.tensor_scalar_mul(wik[:], idt[:], wt[:, k:k + 1])
                wI.append(wik)

            # ---------------- Gram phase ----------------
            with (
                tc.tile_pool(name="gtiles", bufs=1) as tpool,
                tc.tile_pool(name="gstage", bufs=2) as spool,
                tc.tile_pool(name="gpsum", bufs=1, space="PSUM") as ppool,
            ):
                for ti, src in enumerate((zt, zbt)):
                    for half in range(2):
                        for grp, (t0, t1) in ((1, (OWN_TILES, CORE_TILES)),
                                              (0, (0, OWN_TILES))):
                            banks = [ppool.tile([128, 512], f32, tag=f"bank{m}",
                                                name=f"bank_{ti}_{half}_{grp}_{m}")
                                     for m in range(8)]
                            for t in range(t0, t1):
                                tl = tpool.tile([128, D], f32r,
                                                tag=f"in{t % 10}",
                                                name=f"in_{ti}_{half}_{t}")
                                nc.sync.dma_start(
                                    tl[:], src[t * 128:(t + 1) * 128, :].bitcast(f32r))
                                rhs = tl[:, half * 512:half * 512 + 512]
                                for m in range(8):
                                    nc.tensor.matmul(
                                        banks[m][:],
                                        tl[:, m * 128:(m + 1) * 128],
                                        rhs,
                                        start=(t == t0), stop=(t == t1 - 1),
                                        skip_group_check=True)
                            for m in range(8):
                                dst_col = m * 1024 + half * 512
                                if grp == 0:
                                    # own-class Gram -> mats[ti] directly
                                    if m % 2 == 0:
                                        nc.vector.tensor_copy(
                                            mats[ti][:, dst_col:dst_col + 512],
                                            banks[m][:])
                                    else:
                                        nc.scalar.copy(
                                            mats[ti][:, dst_col:dst_col + 512],
                                            banks[m][:])
                                else:
                                    st = spool.tile([128, 512], f32,
                                                    tag=f"st{m % 4}",
                                                    name=f"st_{ti}_{half}_{m}")
                                    if m % 2 == 0:
                                        nc.vector.tensor_copy(st[:], banks[m][:])
                                    else:
                                        nc.scalar.copy(st[:], banks[m][:])
                                    nc.sync.dma_start(
                                        bB[ti * D + m * 128:ti * D + m * 128 + 128,
                                           half * 512:half * 512 + 512], st[:])
                # own-class Grams -> bA for the F collective (pure Grams)
                for ti in range(2):
                    for rb in range(8):
                        nc.sync.dma_start(
                            bA[ti * D + rb * 128:ti * D + rb * 128 + 128, :],
                            mats[ti][:, rb * 1024:rb * 1024 + 1024])

            # ---------------- Collectives ----------------
            nc.gpsimd.collective_compute(
                "AllReduce", mybir.AluOpType.add,
                replica_groups=[[0, 1, 2, 3], [4, 5, 6, 7]],
                ins=[bB.opt()], outs=[rB.opt()])
            nc.gpsimd.collective_compute(
                "AllReduce", mybir.AluOpType.add,
                replica_groups=[list(range(8))],
                ins=[bA.opt()], outs=[rA.opt()])
            nc.gpsimd.collective_compute(
                "AllReduce", mybir.AluOpType.add,
                replica_groups=[list(range(8))],
                ins=[bB.opt()], outs=[rBall.opt()])

            # ---------------- Assembly of B2, B3 ----------------
            with (
                tc.tile_pool(name="atmp", bufs=4) as apool,
                tc.tile_pool(name="apsum", bufs=2, space="PSUM") as appool,
            ):
                # B2 = mat0 + mat1 (+ corrected diag later), via PE identity
                for rb in range(8):
                    for h in range(2):
                        col = rb * 1024 + h * 512
                        ps = appool.tile([128, 512], f32, tag="aps",
                                         name=f"b2ps_{rb}_{h}")
                        nc.tensor.matmul(ps[:], idt[:],
                                         mats[0][:, col:col + 512],
                                         start=True, stop=False,
                                         skip_group_check=True)
                        nc.tensor.matmul(ps[:], idt[:],
                                         mats[1][:, col:col + 512],
                                         start=False, stop=True,
                                         skip_group_check=True)
                        if h == 0:
                            nc.vector.tensor_copy(mats[2][:, col:col + 512], ps[:])
                        else:
                            nc.scalar.copy(mats[2][:, col:col + 512], ps[:])
                # B3 = w0*rB[Z] + w1*rB[Zb] + w2*(rA[Z]+rBall[Z]) + w3*(rA[Zb]+rBall[Zb])
                for rb in range(8):
                    for h in range(2):
                        col = rb * 1024 + h * 512
                        ps = appool.tile([128, 512], f32, tag="aps",
                                         name=f"b3ps_{rb}_{h}")
                        pieces = [(rB, 0, 0), (rB, 1, 1),
                                  (rA, 0, 2), (rBall, 0, 2),
                                  (rA, 1, 3), (rBall, 1, 3)]
                        for pi, (srcb, ti, k) in enumerate(pieces):
                            tmp = apool.tile([128, 512], f32, tag=f"at{pi % 4}",
                                             name=f"b3t_{rb}_{h}_{pi}")
                            nc.sync.dma_start(
                                tmp[:],
                                srcb[ti * D + rb * 128:ti * D + rb * 128 + 128,
                                     h * 512:h * 512 + 512])
                            nc.tensor.matmul(ps[:], wI[k][:],
                                             tmp[:],
                                             start=(pi == 0), stop=(pi == 5),
                                             skip_group_check=True)
                        if h == 0:
                            nc.vector.tensor_copy(mats[3][:, col:col + 512], ps[:])
                        else:
                            nc.scalar.copy(mats[3][:, col:col + 512], ps[:])
                # diag adds: B_m[rb-block diagonal 128-chunk] += diags[m]
                for m in range(4):
                    for rb in range(8):
                        col = rb * 1024 + rb * 128
                        nc.vector.tensor_add(
                            mats[m][:, col:col + 128],
                            mats[m][:, col:col + 128],
                            dg[:, m * 128:(m + 1) * 128])

            # ---------------- logdet phase ----------------
            with (
                tc.tile_pool(name="lwork", bufs=2) as lpool,
                tc.tile_pool(name="lpsum", bufs=2, space="PSUM") as lppool,
                tc.tile_pool(name="piv", bufs=1) as pvpool,
            ):
                pivs = pvpool.tile([128, 8 * 32 * 4], f32, name="pivs")
                for k in range(8):
                    cascb = pvpool.tile([128, 128], f32, tag="casc",
                                        bufs=2, name=f"casc_{k}")
                    for m in range(4):
                        mat = mats[m]

                        def blk(rb, c0, w):
                            return mat[:, rb * 1024 + c0:rb * 1024 + c0 + w]

                        S = blk(k, k * 128, 128)  # [128,128] diag block
                        # --- NS-128: X = inv(S) ---
                        X = lpool.tile([128, 128], f32, tag=f"X{m}",
                                       name=f"X_{k}_{m}")
                        nc.vector.tensor_scalar_mul(X[:], idt[:],
                                                    alp[:, m:m + 1])
                        for it in range(NS128_ITERS):
                            Yp = lppool.tile([128, 128], f32, tag="Yp",
                                             name=f"Yp_{k}_{m}_{it}")
                            nc.tensor.matmul(Yp[:], S, X[:], start=True,
                                             stop=True, skip_group_check=True)
                            T = lpool.tile([128, 128], f32, tag="T",
                                           name=f"T_{k}_{m}_{it}")
                            nc.vector.scalar_tensor_tensor(
                                T[:], Yp[:], -1.0, i2[:], AL.mult, AL.add)
                            X2 = lppool.tile([128, 128], f32, tag="Yp",
                                             name=f"X2_{k}_{m}_{it}")
                            nc.tensor.matmul(X2[:], X[:], T[:], start=True,
                                             stop=True, skip_group_check=True)
                            nc.scalar.copy(X[:], X2[:])

                        # --- panel + trailing update (stages < 7) ---
                        if k < 7:
                            wspan = (7 - k) * 128
                            rowp = blk(k, (k + 1) * 128, wspan)
                            Wt = lpool.tile([128, 896], f32, tag="Wt",
                                            name=f"Wt_{k}_{m}")
                            for c0 in range(0, wspan, 512):
                                w = min(512, wspan - c0)
                                Wp = lppool.tile([128, 512], f32, tag="Wp",
                                                 name=f"Wp_{k}_{m}_{c0}")
                                nc.tensor.matmul(Wp[:, :w], X[:],
                                                 rowp[:, c0:c0 + w],
                                                 start=True, stop=True,
                                                 skip_group_check=True)
                                nc.vector.tensor_scalar_mul(
                                    Wt[:, c0:c0 + w], Wp[:, :w], -1.0)
                            for ib in range(k + 1, 8):
                                wi = 1024 - 128 * ib
                                off = (ib - k - 1) * 128
                                tp = lppool.tile([128, 896], f32, tag="tp",
                                                 name=f"tp_{k}_{m}_{ib}")
                                for c0 in range(0, wi, 512):
                                    w = min(512, wi - c0)
                                    nc.tensor.matmul(
                                        tp[:, c0:c0 + w],
                                        Wt[:, off:off + 128],
                                        rowp[:, off + c0:off + c0 + w],
                                        start=True, stop=True,
                                        skip_group_check=True)
                                tgt = blk(ib, 128 * ib, wi)
                                nc.vector.tensor_tensor(
                                    tgt, tgt, tp[:, :wi], AL.add)

                        # --- cascade pieces into cascb[:, m*32:(m+1)*32] ---
                        cc = cascb[:, m * 32:(m + 1) * 32]
                        # (a) A11 = S[0:32,0:32]
                        nc.vector.tensor_copy(cc[0:32, :], S[0:32, 0:32])
                        # (c) XB11 = X[64:96,64:96]
                        nc.vector.tensor_copy(cc[64:96, :], X[64:96, 64:96])
                        # NS32 a: inv(A11), warm from X[0:32,0:32]
                        Xa = lpool.tile([32, 32], f32, tag="Xa",
                                        name=f"Xa_{k}_{m}")
                        nc.vector.tensor_copy(Xa[:], X[0:32, 0:32])
                        for it in range(NS32_ITERS):
                            yp = lppool.tile([32, 32], f32, tag="Yp",
                                             name=f"ya_{k}_{m}_{it}")
                            nc.tensor.matmul(yp[:], S[0:32, 0:32], Xa[:],
                                             start=True, stop=True,
                                             skip_group_check=True)
                            t3 = lpool.tile([32, 32], f32, tag="t3",
                                            name=f"ta_{k}_{m}_{it}")
                            nc.vector.scalar_tensor_tensor(
                                t3[:], yp[:], -1.0, i2[0:32, 0:32],
                                AL.mult, AL.add)
                            x2 = lppool.tile([32, 32], f32, tag="Yp",
                                             name=f"xa2_{k}_{m}_{it}")
                            nc.tensor.matmul(x2[:], Xa[:], t3[:], start=True,
                                             stop=True, skip_group_check=True)
                            nc.scalar.copy(Xa[:], x2[:])
                        # SchurA = S[32:64,32:64] - A21 Xa A12 -> cc[32:64]
                        t1p = lppool.tile([32, 32], f32, tag="Yp",
                                          name=f"t1a_{k}_{m}")
                        nc.tensor.matmul(t1p[:], Xa[:], S[0:32, 32:64],
                                         start=True, stop=True,
                                         skip_group_check=True)
                        t1s = lpool.tile([32, 32], f32, tag="t3",
                                         name=f"t1as_{k}_{m}")
                        nc.scalar.copy(t1s[:], t1p[:])
                        t2p = lppool.tile([128, 32], f32, tag="Yp",
                                          name=f"t2a_{k}_{m}")
                        nc.tensor.matmul(t2p[32:64, :], S[0:32, 32:64], t1s[:],
                                         start=True, stop=True,
                                         tile_position=(0, 32),
                                         skip_group_check=True)
                        nc.vector.scalar_tensor_tensor(
                            cc[32:64, :], t2p[32:64, :], -1.0, S[32:64, 32:64],
                            AL.mult, AL.add)
                        # NS32 b: inv(XB11), warm from S[64:96,64:96]
                        Xb = lpool.tile([128, 32], f32, tag="Xb",
                                        name=f"Xb_{k}_{m}")
                        nc.vector.tensor_copy(Xb[64:96, :], S[64:96, 64:96])
                        for it in range(NS32_ITERS):
                            yp = lppool.tile([128, 32], f32, tag="Yp",
                                             name=f"yb_{k}_{m}_{it}")
                            nc.tensor.matmul(yp[64:96, :], X[64:96, 64:96],
                                             Xb[64:96, :], start=True,
                                             stop=True, tile_position=(64, 64),
                                             skip_group_check=True)
                            t3 = lpool.tile([128, 32], f32, tag="t3b",
                                            name=f"tb_{k}_{m}_{it}")
                            nc.vector.scalar_tensor_tensor(
                                t3[64:96, :], yp[64:96, :], -1.0,
                                i2[64:96, 64:96], AL.mult, AL.add)
                            x2 = lppool.tile([128, 32], f32, tag="Yp",
                                             name=f"xb2_{k}_{m}_{it}")
                            nc.tensor.matmul(x2[64:96, :], Xb[64:96, :],
                                             t3[64:96, :], start=True,
                                             stop=True, tile_position=(64, 64),
                                             skip_group_check=True)
                            nc.scalar.copy(Xb[64:96, :], x2[64:96, :])
                        # SchurXB = X[96:128,96:128] - XB21 Xb XB12 -> cc[96:128]
                        u1p = lppool.tile([128, 32], f32, tag="Yp",
                                          name=f"u1_{k}_{m}")
                        nc.tensor.matmul(u1p[64:96, :], Xb[64:96, :],
                                         X[64:96, 96:128], start=True,
                                         stop=True, tile_position=(64, 64),
                                         skip_group_check=True)
                        u1s = lpool.tile([128, 32], f32, tag="t3b",
                                         name=f"u1s_{k}_{m}")
                        nc.scalar.copy(u1s[64:96, :], u1p[64:96, :])
                        u2p = lppool.tile([128, 32], f32, tag="Yp",
                                          name=f"u2_{k}_{m}")
                        nc.tensor.matmul(u2p[96:128, :], X[64:96, 96:128],
                                         u1s[64:96, :], start=True, stop=True,
                                         tile_position=(64, 96),
                                         skip_group_check=True)
                        nc.vector.scalar_tensor_tensor(
                            cc[96:128, :], u2p[96:128, :], -1.0,
                            X[96:128, 96:128], AL.mult, AL.add)

                    # --- batched pivot loop over cascb [128, 128] ---
                    b1 = pvpool.tile([128, 128], f32, tag="b1", name=f"b1_{k}")
                    b1t = pvpool.tile([128, 128], f32, tag="b1t",
                                      name=f"b1t_{k}")
                    wv = pvpool.tile([128, 4], f32, tag="wv", name=f"wv_{k}")
                    for j in range(32):
                        # v broadcast: b1[:, g*32+f] = cascb[:, g*32+j]
                        nc.vector.tensor_copy(
                            b1[:].rearrange("p (a b) -> p a b", a=4),
                            cascb[:, j::32].broadcast_to([128, 4, 32]))
                        nc.vector.transpose(b1t[:], b1[:])
                        # w = v / p  ([128,4] strided col slices)
                        vs = cascb[:, j::32]
                        ps_ = b1t[:, j::32]
                        nc.vector.reciprocal(wv[:], ps_)
                        nc.vector.tensor_tensor(wv[:], vs, wv[:], AL.mult)
                        # record pivots
                        nc.vector.tensor_copy(
                            pivs[:, (k * 32 + j) * 4:(k * 32 + j) * 4 + 4], ps_)
                        if j < 31:
                            # M = b1t * broadcast(w); cascb -= M
                            M = pvpool.tile([128, 128], f32, tag="Mt",
                                            name=f"M_{k}_{j}")
                            jj = j + 1
                            nc.vector.tensor_tensor(
                                M[:].rearrange("p (a b) -> p a b", a=4)[:, :, jj:],
                                b1t[:].rearrange("p (a b) -> p a b", a=4)[:, :, jj:],
                                wv[:].broadcast_to([128, 4, 32])[:, :, jj:],
                                AL.mult)
                            cv = cascb[:].rearrange("p (a b) -> p a b", a=4)[:, :, jj:]
                            nc.vector.tensor_tensor(
                                cv, cv,
                                M[:].rearrange("p (a b) -> p a b", a=4)[:, :, jj:],
                                AL.subtract)

                # --- final: logs, sums, sign-combine, output ---
                lnp = pvpool.tile([128, 8 * 32 * 4], f32, name="lnp")
                nc.scalar.activation(lnp[:], pivs[:], AF.Ln)
                lnsum = pvpool.tile([128, 4], f32, name="lnsum")
                for m in range(4):
                    nc.vector.tensor_reduce(lnsum[:, m:m + 1],
                                            lnp[:, m::4],
                                            mybir.AxisListType.X, AL.add)
                tps = lppool.tile([4, 128], f32, tag="Wp", name="tps")
                nc.tensor.transpose(tps[:], lnsum[:], idt[:])
                tss = pvpool.tile([4, 128], f32, name="tss")
                nc.vector.tensor_copy(tss[:], tps[:])
                r1 = pvpool.tile([4, 1], f32, name="r1")
                r2 = pvpool.tile([4, 1], f32, name="r2")
                nc.vector.tensor_reduce(r1[:], tss[:, 0:64], mybir.AxisListType.X, AL.add)
                nc.vector.tensor_reduce(r2[:], tss[:, 64:128], mybir.AxisListType.X, AL.add)
                out4 = pvpool.tile([4, 1], f32, name="out4")
                nc.vector.tensor_tensor(out4[:], r1[:], r2[:], AL.subtract)
                nc.vector.tensor_scalar_mul(out4[:], out4[:], 1.0 / 32.0)
                nc.sync.dma_start(lds_out[:, :], out4[:])
    nc.compile()
    return nc
